# revision 1
# baseline (speedup 1.0000x reference)
"""Distributed Trainium2 Bass kernel for the phasor attention problem
(nn_Attention_17798344475248).

Sharding: 8 cores = 2 batches x 4 head-groups (2 heads each). Each core
computes its batch's Q/K/V projections for its 2 heads, phasor attention,
and a partial final-dense output; partials are summed with 4 pipelined
4-rank ReduceScatters per batch group; each core finishes atan2 on its
4x64-row slices of the output.

Math notes (vs reference.py):
- phasor_encode(phasor_act(z)) == z/|z|  -> normalize instead of atan2+cos/sin
- softmax max-subtract and sum-normalization cancel in the final angle
  (positive per-row scale), so softmax reduces to exp(s/d)
- complex bias (ones) is folded in as K=1 outer-product matmuls
- f32r (reduced-mantissa, full-speed) is used where errors attenuate
  (Q/K projections, scores); f32 where errors reach the output branch cut
  (V projection, PV, final dense)
- all phasor encodes (the only Sin-set ACT work) run in one phase at the
  start, staged via DRAM, so the ACT table set never thrashes afterwards
"""
import sys

sys.path.insert(0, "/opt/trn_rl_repo")

import numpy as np

import concourse.bass as bass
import concourse.tile as tile
from concourse import bacc, mybir
from concourse.bass_utils import run_bass_kernel_spmd
from concourse.masks import make_identity

F32 = mybir.dt.float32
F32R = mybir.dt.float32r
AF = mybir.ActivationFunctionType
ALU = mybir.AluOpType
PI = float(np.pi)

B, T, D, H = 2, 1024, 512, 8
P = 128
DS = D // P          # 4 partition-slices of the model dim
KVCH = 512           # keyvalue chunk width (t)
QCH = 256            # query chunk width (t)
N_CORES = 8
HPC = 2              # heads per core


def _norm_pair(nc, pools, re_ps, im_ps, re_out, im_out, width):
    """Normalize complex (re,im) [128,width] from PSUM to unit modulus:
    n = exp(-0.5*ln(re^2+im^2)); out = (re*n, im*n)."""
    nt = pools["nt"]
    t1 = nt.tile([P, width], F32, tag="nt")
    nc.scalar.activation(t1[:], re_ps[:], AF.Square, bias=0.0, scale=1.0)
    t2 = nt.tile([P, width], F32, tag="nt")
    nc.scalar.activation(t2[:], im_ps[:], AF.Square, bias=0.0, scale=1.0)
    m = nt.tile([P, width], F32, tag="nt")
    nc.vector.tensor_tensor(m[:], t1[:], t2[:], ALU.add)
    sq = nt.tile([P, width], F32, tag="nt")
    nc.scalar.activation(sq[:], m[:], AF.Sqrt, bias=0.0, scale=1.0)
    n = nt.tile([P, width], F32, tag="nt")
    nc.vector.reciprocal_approx_fast(n[:], sq[:])
    nc.vector.tensor_tensor(re_out, re_ps[:], n[:], ALU.mult)
    nc.vector.tensor_tensor(im_out, im_ps[:], n[:], ALU.mult)


def build(debug=False):
    nc = bacc.Bacc("TRN2", target_bir_lowering=False, debug=False,
                   num_devices=N_CORES)
    cpi2 = nc.alloc_sbuf_tensor("const-f32-pi2", [P, 1], F32)
    nc.gpsimd.memset(cpi2.ap(), PI / 2)
    nc.const_aps.aps[(F32, PI / 2)] = cpi2.ap()
    nc.all_engine_barrier()

    # ---- I/O ----
    QUERY = nc.dram_tensor("query", [T, D], F32, kind="ExternalInput")
    KEYVALUE = nc.dram_tensor("keyvalue", [T, D], F32, kind="ExternalInput")
    WQ = nc.dram_tensor("wq", [HPC, D, D], F32, kind="ExternalInput")
    WK = nc.dram_tensor("wk", [HPC, D, D], F32, kind="ExternalInput")
    WV = nc.dram_tensor("wv", [HPC, D, D], F32, kind="ExternalInput")
    BQ = nc.dram_tensor("bq", [HPC, D], F32, kind="ExternalInput")
    BK = nc.dram_tensor("bk", [HPC, D], F32, kind="ExternalInput")
    BV = nc.dram_tensor("bv", [HPC, D], F32, kind="ExternalInput")
    WO = nc.dram_tensor("wo", [HPC * D, D], F32, kind="ExternalInput")
    BO = nc.dram_tensor("bo", [D], F32, kind="ExternalInput")
    OUT = nc.dram_tensor("out", [T // 4, D], F32, kind="ExternalOutput")

    with tile.TileContext(nc) as tc:
        import contextlib
        with contextlib.ExitStack() as ctx:
            pools = {}
            for name, bufs, space in [
                ("persist", 1, "SBUF"), ("raw", 4, "SBUF"), ("nt", 6, "SBUF"),
                ("w", 1, "SBUF"), ("wr", 4, "SBUF"),
                ("browf", 1, "SBUF"), ("browr", 3, "SBUF"),
                ("kvr", 2, "SBUF"), ("qenc", 2, "SBUF"),
                ("kt", 2, "SBUF"), ("v", 2, "SBUF"),
                ("qt", 2, "SBUF"), ("p", 1, "SBUF"), ("oh", 2, "SBUF"),
                ("z", 4, "SBUF"), ("ps", 8, "PSUM"), ("dram", 1, "DRAM"),
            ]:
                pools[name] = ctx.enter_context(
                    tc.tile_pool(name=name, bufs=bufs, space=space))

            persist = pools["persist"]
            ident = persist.tile([P, P], F32, tag="ident")
            make_identity(nc, ident[:])

            # ---- small constant rows ----
            ones512_f = persist.tile([1, KVCH], F32, tag="ones512f")
            nc.vector.memset(ones512_f[:], 1.0)
            ones512_r = persist.tile([1, KVCH], F32R, tag="ones512r")
            nc.vector.tensor_copy(ones512_r[:], ones512_f[:])
            ones_row_f = ones512_f[0:1, :P]                  # [1,128] f32 = 1.0
            quarter_f = persist.tile([1, P], F32, tag="quarterf")
            nc.vector.memset(quarter_f[:], 0.125)  # bo split over 4 cores x 2 heads
            quarter_r = persist.tile([1, P], F32R, tag="quarterr")
            nc.vector.tensor_copy(quarter_r[:], quarter_f[:])
            bo_r = persist.tile([1, D], F32R, tag="bor")
            bo_f = pools["browf"].tile([1, D], F32, tag="browf", name="bo_f")
            nc.sync.dma_start(bo_f[:], BO[:][None, :])
            nc.vector.tensor_copy(bo_r[:], bo_f[:])

            # ---- DRAM staging ----
            dram = pools["dram"]
            zbs = [dram.tile([KVCH, D], F32, name=f"zb{q}") for q in range(4)]
            rs_outs = [dram.tile([P, D], F32, name=f"rsout{q}") for q in range(4)]
            qe_cos = dram.tile([P, DS, T], F32R, name="qe_cos")
            qe_sin = dram.tile([P, DS, T], F32R, name="qe_sin")
            kve_cos_r = dram.tile([P, DS, T], F32R, name="kve_cos_r")
            kve_sin_r = dram.tile([P, DS, T], F32R, name="kve_sin_r")

            # ================= Phase 1: all phasor encodes =================
            # (the only Sin-set ACT work in the kernel)
            for src_dram, is_q in ((KEYVALUE, False), (QUERY, True)):
                for ch in range(T // KVCH):
                    raw_tiles = []
                    for ts in range(KVCH // P):
                        rt = pools["raw"].tile([P, D], F32, tag="raw")
                        nc.sync.dma_start(
                            rt[:],
                            src_dram[ch * KVCH + ts * P: ch * KVCH + (ts + 1) * P, :])
                        raw_tiles.append(rt)
                    cos_t = pools["kvr"].tile([P, DS, KVCH], F32R, tag="kvr",
                                              name=f"enc_cos_{is_q}_{ch}")
                    sin_t = pools["kvr"].tile([P, DS, KVCH], F32R, tag="kvr",
                                              name=f"enc_sin_{is_q}_{ch}")
                    for ds in range(DS):
                        pt = pools["ps"].tile([P, KVCH], F32, tag="ps")
                        for ts in range(KVCH // P):
                            nc.tensor.transpose(
                                pt[:, ts * P:(ts + 1) * P],
                                raw_tiles[ts][:, ds * P:(ds + 1) * P], ident[:])
                        nc.scalar.activation(sin_t[:, ds, :], pt[:], AF.Sin,
                                             bias=0.0, scale=PI)
                        ab = pools["nt"].tile([P, KVCH], F32, tag="nt")
                        nc.scalar.activation(ab[:], pt[:], AF.Abs, bias=0.0, scale=1.0)
                        nc.scalar.activation(cos_t[:, ds, :], ab[:], AF.Sin,
                                             bias=PI / 2, scale=-PI)
                    chsl = slice(ch * KVCH, (ch + 1) * KVCH)
                    if is_q:
                        nc.sync.dma_start(qe_cos[:, :, chsl], cos_t[:])
                        nc.sync.dma_start(qe_sin[:, :, chsl], sin_t[:])
                    else:
                        nc.sync.dma_start(kve_cos_r[:, :, chsl], cos_t[:])
                        nc.sync.dma_start(kve_sin_r[:, :, chsl], sin_t[:])

            # ================= Phase 2: per-head attention =================
            for h in range(HPC):
                def _bias_row_r(SRC, name):
                    bf = pools["browf"].tile([1, D], F32, tag="browf", name=name + "_f")
                    nc.sync.dma_start(bf[:], SRC[h][None, :])
                    br = pools["browr"].tile([1, D], F32R, tag="browr", name=name + "_r")
                    nc.vector.tensor_copy(br[:], bf[:])
                    return br
                bq_row = _bias_row_r(BQ, f"bq{h}")
                bk_row = _bias_row_r(BK, f"bk{h}")
                bv_row = _bias_row_r(BV, f"bv{h}")

                # ---- weights ----
                wq_r = pools["wr"].tile([P, DS, D], F32R, tag="wr")
                wk_r = pools["wr"].tile([P, DS, D], F32R, tag="wr")
                wv_r = pools["wr"].tile([P, DS, D], F32R, tag="wr")
                wo_r = pools["wr"].tile([P, DS, D], F32R, tag="wr")
                for W_ap, w_r in ((WQ[h], wq_r), (WK[h], wk_r), (WV[h], wv_r),
                                  (WO[h * D:(h + 1) * D, :], wo_r)):
                    wf = pools["w"].tile([P, DS, D], F32, tag="wf")
                    nc.sync.dma_start(
                        wf[:], W_ap.rearrange("(o p) D -> p o D", p=P))
                    nc.vector.tensor_copy(w_r[:], wf[:])

                # ---- persistent per-head tensors ----
                kt_re = pools["kt"].tile([P, DS, T], F32R, tag="kt")
                kt_im = pools["kt"].tile([P, DS, T], F32R, tag="kt")
                v_re = pools["v"].tile([P, T // P, D], F32R, tag="v")
                v_im = pools["v"].tile([P, T // P, D], F32R, tag="v")

                # ======== KV pass: K^T (f32r) and V (f32) ========
                for ch in range(T // KVCH):
                    chsl = slice(ch * KVCH, (ch + 1) * KVCH)
                    kv_cos_r = pools["kvr"].tile([P, DS, KVCH], F32R, tag="kvr",
                                                 name=f"kvcr_{h}_{ch}")
                    kv_sin_r = pools["kvr"].tile([P, DS, KVCH], F32R, tag="kvr",
                                                 name=f"kvsr_{h}_{ch}")
                    nc.sync.dma_start(kv_cos_r[:], kve_cos_r[:, :, chsl])
                    nc.sync.dma_start(kv_sin_r[:], kve_sin_r[:, :, chsl])

                    # V projection (f32)
                    for ts in range(KVCH // P):
                        pre = pools["ps"].tile([P, D], F32, tag="ps")
                        pim = pools["ps"].tile([P, D], F32, tag="ps")
                        for do in range(DS):
                            nc.tensor.matmul(
                                pre[:], lhsT=kv_cos_r[:, do, ts * P:(ts + 1) * P],
                                rhs=wv_r[:, do, :], start=(do == 0), stop=False)
                        nc.tensor.matmul(
                            pre[:], lhsT=ones512_r[0:1, :P], rhs=bv_row[:],
                            start=False, stop=True)
                        for do in range(DS):
                            nc.tensor.matmul(
                                pim[:], lhsT=kv_sin_r[:, do, ts * P:(ts + 1) * P],
                                rhs=wv_r[:, do, :], start=(do == 0),
                                stop=(do == DS - 1))
                        trow = ch * (KVCH // P) + ts
                        _norm_pair(nc, pools, pre, pim,
                                   v_re[:, trow, :], v_im[:, trow, :], D)

                    # K projection (f32r): K^T [D', t]
                    for dso in range(DS):
                        pre = pools["ps"].tile([P, KVCH], F32, tag="ps")
                        pim = pools["ps"].tile([P, KVCH], F32, tag="ps")
                        for do in range(DS):
                            nc.tensor.matmul(
                                pre[:], lhsT=wk_r[:, do, dso * P:(dso + 1) * P],
                                rhs=kv_cos_r[:, do, :], start=(do == 0), stop=False)
                        nc.tensor.matmul(
                            pre[:], lhsT=bk_row[0:1, dso * P:(dso + 1) * P],
                            rhs=ones512_r[:], start=False, stop=True)
                        for do in range(DS):
                            nc.tensor.matmul(
                                pim[:], lhsT=wk_r[:, do, dso * P:(dso + 1) * P],
                                rhs=kv_sin_r[:, do, :], start=(do == 0),
                                stop=(do == DS - 1))
                        _norm_pair(nc, pools, pre, pim,
                                   kt_re[:, dso, chsl], kt_im[:, dso, chsl], KVCH)

                # ======== Q pass per 256-chunk ========
                for ch in range(T // QCH):
                    qsl = slice(ch * QCH, (ch + 1) * QCH)
                    q_cos_r = pools["qenc"].tile([P, DS, QCH], F32R, tag="qenc",
                                                 name=f"qc_{h}_{ch}")
                    q_sin_r = pools["qenc"].tile([P, DS, QCH], F32R, tag="qenc",
                                                 name=f"qs_{h}_{ch}")
                    nc.sync.dma_start(q_cos_r[:], qe_cos[:, :, qsl])
                    nc.sync.dma_start(q_sin_r[:], qe_sin[:, :, qsl])

                    qt_re = pools["qt"].tile([P, DS, QCH], F32R, tag="qt")
                    qt_im = pools["qt"].tile([P, DS, QCH], F32R, tag="qt")
                    for dso in range(DS):
                        pre = pools["ps"].tile([P, QCH], F32, tag="ps")
                        pim = pools["ps"].tile([P, QCH], F32, tag="ps")
                        for do in range(DS):
                            nc.tensor.matmul(
                                pre[:], lhsT=wq_r[:, do, dso * P:(dso + 1) * P],
                                rhs=q_cos_r[:, do, :], start=(do == 0), stop=False)
                        nc.tensor.matmul(
                            pre[:], lhsT=bq_row[0:1, dso * P:(dso + 1) * P],
                            rhs=ones512_r[0:1, :QCH], start=False, stop=True)
                        for do in range(DS):
                            nc.tensor.matmul(
                                pim[:], lhsT=wq_r[:, do, dso * P:(dso + 1) * P],
                                rhs=q_sin_r[:, do, :], start=(do == 0),
                                stop=(do == DS - 1))
                        _norm_pair(nc, pools, pre, pim,
                                   qt_re[:, dso, :], qt_im[:, dso, :], QCH)

                    # scores + exp -> P^T [Tkv, tq-chunk]
                    pt_all = pools["p"].tile([P, T // P, QCH], F32R, tag="p")
                    for to in range(T // P):
                        ps_s = pools["ps"].tile([P, QCH], F32, tag="ps")
                        for do in range(DS):
                            nc.tensor.matmul(
                                ps_s[:], lhsT=kt_re[:, do, to * P:(to + 1) * P],
                                rhs=qt_re[:, do, :], start=(do == 0), stop=False)
                        for do in range(DS):
                            nc.tensor.matmul(
                                ps_s[:], lhsT=kt_im[:, do, to * P:(to + 1) * P],
                                rhs=qt_im[:, do, :], start=False,
                                stop=(do == DS - 1))
                        nc.scalar.activation(pt_all[:, to, :], ps_s[:], AF.Exp,
                                             bias=0.0, scale=1.0 / D)

                    # PV (f32): O^T [D', tq-chunk], two groups of 2 D'-slices
                    oh_re = pools["oh"].tile([P, DS, QCH], F32R, tag="oh")
                    oh_im = pools["oh"].tile([P, DS, QCH], F32R, tag="oh")
                    for grp in range(2):
                        ps_tiles = {}
                        for dso in (2 * grp, 2 * grp + 1):
                            ps_tiles[(dso, 0)] = pools["ps"].tile(
                                [P, QCH], F32, tag="ps", name=f"pv_{h}_{ch}_{dso}_re")
                            ps_tiles[(dso, 1)] = pools["ps"].tile(
                                [P, QCH], F32, tag="ps", name=f"pv_{h}_{ch}_{dso}_im")
                        for to in range(T // P):
                            for dso in (2 * grp, 2 * grp + 1):
                                nc.tensor.matmul(
                                    ps_tiles[(dso, 0)][:],
                                    lhsT=v_re[:, to, dso * P:(dso + 1) * P],
                                    rhs=pt_all[:, to, :], start=(to == 0),
                                    stop=(to == T // P - 1))
                                nc.tensor.matmul(
                                    ps_tiles[(dso, 1)][:],
                                    lhsT=v_im[:, to, dso * P:(dso + 1) * P],
                                    rhs=pt_all[:, to, :], start=(to == 0),
                                    stop=(to == T // P - 1))
                        for dso in (2 * grp, 2 * grp + 1):
                            _norm_pair(nc, pools, ps_tiles[(dso, 0)],
                                       ps_tiles[(dso, 1)],
                                       oh_re[:, dso, :], oh_im[:, dso, :], QCH)

                    # final dense partial: Z [tq, D'] += Ohat^T.T @ wo_h
                    for ts in range(QCH // P):
                        pzre = pools["ps"].tile([P, D], F32, tag="ps")
                        pzim = pools["ps"].tile([P, D], F32, tag="ps")
                        for do in range(DS):
                            nc.tensor.matmul(
                                pzre[:], lhsT=oh_re[:, do, ts * P:(ts + 1) * P],
                                rhs=wo_r[:, do, :], start=(do == 0), stop=False)
                        nc.tensor.matmul(
                            pzre[:], lhsT=quarter_r[:], rhs=bo_r[:],
                            start=False, stop=True)
                        for do in range(DS):
                            nc.tensor.matmul(
                                pzim[:], lhsT=oh_im[:, do, ts * P:(ts + 1) * P],
                                rhs=wo_r[:, do, :], start=(do == 0),
                                stop=(do == DS - 1))
                        # zb row layout (per tq quarter qq of 256 rows):
                        #   qq*512 + r*128 + c*64 + i, r = rank-block, c = 0 re / 1 im
                        tq0 = ch * QCH + ts * P
                        qq = tq0 // 256
                        u0 = tq0 % 256        # 0 or 128 within the quarter
                        r0 = u0 // 64
                        zre_sb = pools["z"].tile([P, D], F32, tag="z")
                        zim_sb = pools["z"].tile([P, D], F32, tag="z")
                        if h == 0:
                            nc.vector.tensor_copy(zre_sb[:], pzre[:])
                            nc.vector.tensor_copy(zim_sb[:], pzim[:])
                        else:
                            # fuse head-0 partial add into the PSUM drain
                            h0re = pools["z"].tile([P, D], F32, tag="z",
                                                   name=f"h0re_{ch}_{ts}")
                            h0im = pools["z"].tile([P, D], F32, tag="z",
                                                   name=f"h0im_{ch}_{ts}")
                            for half in range(2):
                                r_ = r0 + half
                                src = slice(half * 64, (half + 1) * 64)
                                nc.sync.dma_start(h0re[src, :],
                                                  zbs[qq][r_ * P: r_ * P + 64, :])
                                nc.sync.dma_start(h0im[src, :],
                                                  zbs[qq][r_ * P + 64: r_ * P + 128, :])
                            nc.vector.tensor_tensor(zre_sb[:], pzre[:], h0re[:],
                                                    ALU.add)
                            nc.vector.tensor_tensor(zim_sb[:], pzim[:], h0im[:],
                                                    ALU.add)
                        for half in range(2):  # two 64-row rank blocks per tile
                            r_ = r0 + half
                            dst_re = zbs[qq][r_ * P: r_ * P + 64, :]
                            dst_im = zbs[qq][r_ * P + 64: r_ * P + 128, :]
                            src = slice(half * 64, (half + 1) * 64)
                            nc.sync.dma_start(dst_re, zre_sb[src, :])
                            nc.sync.dma_start(dst_im, zim_sb[src, :])

                    # fire the sub-ReduceScatter for this tq quarter once the
                    # second head's accumulation for it has landed
                    if h == HPC - 1:
                        qq = ch  # QCH == 256 -> chunk == quarter
                        nc.gpsimd.collective_compute(
                            "ReduceScatter", ALU.add,
                            replica_groups=[[0, 1, 2, 3], [4, 5, 6, 7]],
                            ins=[zbs[qq].opt()],
                            outs=[rs_outs[qq].opt()],
                        )

            # ======== atan2(zim, zre)/pi, quarters batched in pairs ========
            for pp in range(2):
                qa, qb = 2 * pp, 2 * pp + 1
                zre_t = pools["z"].tile([P, D], F32, tag="z", name=f"zre{pp}")
                nc.sync.dma_start(zre_t[0:64, :], rs_outs[qa][0:64, :])
                nc.sync.dma_start(zre_t[64:128, :], rs_outs[qb][0:64, :])
                zim_t = pools["z"].tile([P, D], F32, tag="z", name=f"zim{pp}")
                nc.sync.dma_start(zim_t[0:64, :], rs_outs[qa][64:128, :])
                nc.sync.dma_start(zim_t[64:128, :], rs_outs[qb][64:128, :])
                zre, zim = zre_t[:, :], zim_t[:, :]
                nt = pools["nt"]

                def ft(nm, pp=pp):
                    return nt.tile([P, D], F32, tag="nt", name=f"{nm}{pp}")
                t1 = ft("f1")
                nc.scalar.activation(t1[:], zre, AF.Square, bias=0.0, scale=1.0)
                t2 = ft("f2")
                nc.vector.tensor_tensor(t2[:], zim, zim, ALU.mult)
                m = ft("f3")
                nc.vector.tensor_tensor(m[:], t1[:], t2[:], ALU.add)
                az = ft("f5")
                nc.scalar.activation(az[:], m[:], AF.Sqrt, bias=0.0, scale=1.0)
                den1 = ft("f6")
                nc.vector.tensor_tensor(den1[:], az[:], zre, ALU.add)
                r1 = ft("f7")
                nc.vector.reciprocal_approx_fast(r1[:], den1[:])
                ta0 = ft("f8")
                nc.vector.tensor_tensor(ta0[:], zim, r1[:], ALU.mult)
                ta = ft("f9")
                nc.vector.tensor_scalar(ta[:], ta0[:], 1e8, -1e8, ALU.min, ALU.max)
                num2 = ft("fa")
                nc.vector.tensor_tensor(num2[:], az[:], zre, ALU.subtract)
                r2 = ft("fb")
                nc.vector.reciprocal_approx_fast(r2[:], zim)
                tb0 = ft("fc")
                nc.vector.tensor_tensor(tb0[:], num2[:], r2[:], ALU.mult)
                tb = ft("fd")
                nc.vector.tensor_scalar(tb[:], tb0[:], 1e8, -1e8, ALU.min, ALU.max)
                ata = ft("fe")
                nc.scalar.activation(ata[:], ta[:], AF.Arctan, bias=0.0, scale=1.0)
                atb = ft("ff")
                nc.scalar.activation(atb[:], tb[:], AF.Arctan, bias=0.0, scale=1.0)
                mask = ft("fg")
                nc.vector.tensor_scalar(mask[:], zre, 0.0, None, ALU.is_ge)
                dsel = ft("fh")
                nc.vector.tensor_tensor(dsel[:], ata[:], atb[:], ALU.subtract)
                md = ft("fi")
                nc.vector.tensor_tensor(md[:], mask[:], dsel[:], ALU.mult)
                sel = ft("fj")
                nc.vector.tensor_tensor(sel[:], atb[:], md[:], ALU.add)
                outt = ft("fk")
                nc.vector.tensor_scalar(outt[:], sel[:], 2.0 / PI, None, ALU.mult)
                nc.sync.dma_start(OUT[qa * 64:(qa + 1) * 64, :], outt[0:64, :])
                nc.sync.dma_start(OUT[qb * 64:(qb + 1) * 64, :], outt[64:128, :])

    nc.finalize()
    return nc


_NC_CACHE = {}


def _get_nc():
    if "nc" not in _NC_CACHE:
        _NC_CACHE["nc"] = build()
    return _NC_CACHE["nc"]


def kernel(**inputs):
    query = np.ascontiguousarray(np.asarray(inputs["query"], dtype=np.float32))
    keyvalue = np.ascontiguousarray(np.asarray(inputs["keyvalue"], dtype=np.float32))
    wq = np.asarray(inputs["wq"], dtype=np.float32)
    wk = np.asarray(inputs["wk"], dtype=np.float32)
    wv = np.asarray(inputs["wv"], dtype=np.float32)
    bq = np.asarray(inputs["bq"], dtype=np.float32)
    bk = np.asarray(inputs["bk"], dtype=np.float32)
    bv = np.asarray(inputs["bv"], dtype=np.float32)
    wo = np.asarray(inputs["wo"], dtype=np.float32)
    bo = np.asarray(inputs["bo"], dtype=np.float32)

    in_maps = []
    for c in range(N_CORES):
        b, g = c // 4, c % 4
        h0 = g * HPC
        in_maps.append({
            "query": query[b],
            "keyvalue": keyvalue[b],
            "wq": np.ascontiguousarray(wq[h0:h0 + HPC]),
            "wk": np.ascontiguousarray(wk[h0:h0 + HPC]),
            "wv": np.ascontiguousarray(wv[h0:h0 + HPC]),
            "bq": np.ascontiguousarray(bq[h0:h0 + HPC]),
            "bk": np.ascontiguousarray(bk[h0:h0 + HPC]),
            "bv": np.ascontiguousarray(bv[h0:h0 + HPC]),
            "wo": np.ascontiguousarray(wo[h0 * D:(h0 + HPC) * D]),
            "bo": bo,
        })

    nc = _get_nc()
    res = run_bass_kernel_spmd(nc, in_maps, core_ids=list(range(N_CORES)))
    _NC_CACHE["last_results"] = res
    out = np.empty((B, T, D), np.float32)
    for c in range(N_CORES):
        b, g = c // 4, c % 4
        o = res.results[c]["out"]          # [256, 512]: 4 quarters x 64 rows
        for qq in range(4):
            out[b, qq * 256 + g * 64: qq * 256 + (g + 1) * 64, :] = \
                o[qq * 64:(qq + 1) * 64, :]
    return out



# revision 18
# speedup vs baseline: 1.0309x; 1.0309x over previous
"""Distributed Trainium2 Bass kernel for the phasor attention problem
(nn_Attention_17798344475248).

Sharding: 8 cores = 2 batches x 4 head-groups (2 heads each). Each core
computes its batch's Q/K/V projections for its 2 heads, phasor attention,
and a partial final-dense output; partials are summed with 4 pipelined
4-rank ReduceScatters per batch group; each core finishes atan2 on its
4x64-row slices of the output.

v2 design notes:
- all encodes SBUF-resident (no DRAM staging round trips)
- norm n = exp(-0.5*ln(re^2+im^2)): Ln+Exp live in the same ACT table
  set as the scores' Exp -> no act-table thrashing in steady state
- attention core (kt/qt/v/pt, kv/q encodes, wq/wk/wv) in bf16: scores
  are softmax(x/512)-insensitive; V-path noise attenuates over the
  1024-key bundle. oh bf16 (stationary) x wo f32r (moving).
- Q/K biases folded into the norm (per-partition adds), not PE matmuls
- z accumulated in SBUF across heads; zb written once, ReduceScatter
  fired per tq-quarter as head 1 completes it
- QCH=512 (max moving dim): fewer, bigger instructions everywhere
"""
import sys

sys.path.insert(0, "/opt/trn_rl_repo")

import numpy as np

import concourse.bass as bass
import concourse.tile as tile
from concourse import bacc, mybir
from concourse.bass_utils import run_bass_kernel_spmd
from concourse.masks import make_identity

F32 = mybir.dt.float32
F32R = mybir.dt.float32r
F16 = mybir.dt.float16
AF = mybir.ActivationFunctionType
ALU = mybir.AluOpType
PI = float(np.pi)

B, T, D, H = 2, 1024, 512, 8
P = 128
DS = D // P          # 4 partition-slices of the model dim
CH = 512             # chunk width along t (both kv and q passes)
NCH = T // CH        # 2
N_CORES = 8
HPC = 2              # heads per core


def build(debug=False):
    nc = bacc.Bacc("TRN2", target_bir_lowering=False, debug=False,
                   num_devices=N_CORES)
    cpi2 = nc.alloc_sbuf_tensor("const-f32-pi2", [P, 1], F32)
    nc.gpsimd.memset(cpi2.ap(), PI / 2)
    nc.const_aps.aps[(F32, PI / 2)] = cpi2.ap()
    nc.all_engine_barrier()

    # ---- I/O ----
    QUERY = nc.dram_tensor("query", [T, D], F32, kind="ExternalInput")
    KEYVALUE = nc.dram_tensor("keyvalue", [T, D], F32, kind="ExternalInput")
    WQ = nc.dram_tensor("wq", [HPC, D, D], F32, kind="ExternalInput")
    WK = nc.dram_tensor("wk", [HPC, D, D], F32, kind="ExternalInput")
    WV = nc.dram_tensor("wv", [HPC, D, D], F32, kind="ExternalInput")
    BQ = nc.dram_tensor("bq", [HPC, D], F32, kind="ExternalInput")
    BK = nc.dram_tensor("bk", [HPC, D], F32, kind="ExternalInput")
    BV = nc.dram_tensor("bv", [HPC, D], F32, kind="ExternalInput")
    WO = nc.dram_tensor("wo", [HPC * D, D], F32, kind="ExternalInput")
    BO = nc.dram_tensor("bo", [D], F32, kind="ExternalInput")
    OUT = nc.dram_tensor("out", [T // 4, D], F32, kind="ExternalOutput")

    with tile.TileContext(nc) as tc:
        import contextlib
        with contextlib.ExitStack() as ctx:
            pools = {}
            for name, bufs, space in [
                ("persist", 1, "SBUF"),   # ident, z, bo8, ones, enc, ktv
                ("raw", 4, "SBUF"),
                ("nt", 6, "SBUF"),        # norm temps
                ("wf", 1, "SBUF"),        # f32 weight staging
                ("w", 1, "SBUF"),         # casted weights (per-head, tags)
                ("brow", 2, "SBUF"),      # bias staging rows
                ("qt", 1, "SBUF"), ("pt", 1, "SBUF"), ("oh", 1, "SBUF"),
                ("zs", 4, "SBUF"),        # h1 drain staging
                ("z", 4, "SBUF"),         # atan2 tiles
                ("ps", 8, "PSUM"), ("dram", 1, "DRAM"),
            ]:
                pools[name] = ctx.enter_context(
                    tc.tile_pool(name=name, bufs=bufs, space=space))

            persist = pools["persist"]
            nt = pools["nt"]
            ident = persist.tile([P, P], F32, tag="ident")
            make_identity(nc, ident[:])

            # ---- constant rows ----
            ones_f = persist.tile([1, CH], F32, tag="onesf")
            nc.vector.memset(ones_f[:], 1.0)
            ones_bf = persist.tile([1, CH], F16, tag="onesbf")
            nc.vector.tensor_copy(ones_bf[:], ones_f[:])
            q128_f = persist.tile([1, P], F32, tag="q128f")
            nc.vector.memset(q128_f[:], 0.25)   # bo added once per core, 4 cores
            q128_r = persist.tile([1, P], F32R, tag="q128r")
            nc.vector.tensor_copy(q128_r[:], q128_f[:])
            bo_f = pools["brow"].tile([1, D], F32, tag="brow", name="bo_f")
            nc.sync.dma_start(bo_f[:], BO[:][None, :])
            bo_r = persist.tile([1, D], F32R, tag="bor")
            nc.vector.tensor_copy(bo_r[:], bo_f[:])
            # bo/8 broadcast to all partitions (folded into the h0 z drain)
            ps_bo = pools["ps"].tile([P, D], F32, tag="ps", name="ps_bo")
            nc.tensor.matmul(ps_bo[:], lhsT=q128_r[:], rhs=bo_r[:],
                             start=True, stop=True)
            bo8 = persist.tile([P, D], F32, tag="bo8")
            nc.vector.tensor_copy(bo8[:], ps_bo[:])

            # ---- per-head bias columns [128, DS] (d = dso*128 + p) ----
            bqc, bkc = [], []
            bv_rows = []
            for h in range(HPC):
                t_ = persist.tile([P, DS], F32, tag=f"bqc{h}", name=f"bqc{h}")
                nc.sync.dma_start(t_[:], BQ[h].rearrange("(o p) -> p o", p=P))
                bqc.append(t_)
                t_ = persist.tile([P, DS], F32, tag=f"bkc{h}", name=f"bkc{h}")
                nc.sync.dma_start(t_[:], BK[h].rearrange("(o p) -> p o", p=P))
                bkc.append(t_)
                bvf = pools["brow"].tile([1, D], F32, tag="brow",
                                         name=f"bvf{h}")
                nc.sync.dma_start(bvf[:], BV[h][None, :])
                bvr = persist.tile([1, D], F16, tag=f"bvr{h}")
                nc.vector.tensor_copy(bvr[:], bvf[:])
                bv_rows.append(bvr)

            # ---- weights (per-head tags, bufs=1: h1 load waits h0 users) ----
            def load_w(W_ap, dt, tag):
                wf = pools["wf"].tile([P, DS, D], F32, tag="wf")
                nc.sync.dma_start(wf[:], W_ap.rearrange("(o p) D -> p o D", p=P))
                w = pools["w"].tile([P, DS, D], dt, tag=tag)
                nc.vector.tensor_copy(w[:], wf[:])
                return w

            # ---- persistent state ----
            kv_cos = persist.tile([P, DS, T], F16, tag="kvcos")
            kv_sin = persist.tile([P, DS, T], F16, tag="kvsin")
            q_cos = persist.tile([P, DS, T], F16, tag="qcos")
            q_sin = persist.tile([P, DS, T], F16, tag="qsin")
            kt_re = persist.tile([P, DS, T], F16, tag="ktre")
            kt_im = persist.tile([P, DS, T], F16, tag="ktim")
            v_re = persist.tile([P, T // P, D], F16, tag="vre")
            v_im = persist.tile([P, T // P, D], F16, tag="vim")
            z_re = persist.tile([P, T // P, D], F32, tag="zre")
            z_im = persist.tile([P, T // P, D], F32, tag="zim")

            dram = pools["dram"]
            zbs = [dram.tile([4 * P, D], F32, name=f"zb{q}") for q in range(4)]
            rs_outs = [dram.tile([P, D], F32, name=f"rsout{q}")
                       for q in range(4)]

            # prefetch h0 weights before encode phase floods the queues
            w_cache = {}
            for h in (0,):
                w_cache[("q", h)] = load_w(WQ[h], F16, "wq")
                w_cache[("k", h)] = load_w(WK[h], F16, "wk")
                w_cache[("v", h)] = load_w(WV[h], F16, "wv")
                w_cache[("o", h)] = load_w(WO[h * D:(h + 1) * D, :], F16, "wo")

            # ================= Phase A: all phasor encodes =================
            for src_dram, cos_t, sin_t in ((KEYVALUE, kv_cos, kv_sin),
                                           (QUERY, q_cos, q_sin)):
                for ch in range(NCH):
                    chsl = slice(ch * CH, (ch + 1) * CH)
                    raw_tiles = []
                    for ts in range(CH // P):
                        rt = pools["raw"].tile([P, D], F32, tag="raw")
                        nc.sync.dma_start(
                            rt[:],
                            src_dram[ch * CH + ts * P: ch * CH + (ts + 1) * P, :])
                        raw_tiles.append(rt)
                    for ds in range(DS):
                        pt_ = pools["ps"].tile([P, CH], F32, tag="ps")
                        for ts in range(CH // P):
                            nc.tensor.transpose(
                                pt_[:, ts * P:(ts + 1) * P],
                                raw_tiles[ts][:, ds * P:(ds + 1) * P], ident[:])
                        nc.scalar.activation(sin_t[:, ds, chsl], pt_[:], AF.Sin,
                                             bias=0.0, scale=PI)
                        ab = nt.tile([P, CH], F32, tag="nt")
                        nc.scalar.activation(ab[:], pt_[:], AF.Abs,
                                             bias=0.0, scale=1.0)
                        nc.scalar.activation(cos_t[:, ds, chsl], ab[:], AF.Sin,
                                             bias=PI / 2, scale=-PI)

            # ---- norm helper: out = z/|z| (+ optional bias col on re) ----
            # Pool (gpsimd) cannot access PSUM: it only gets the SBUF add.
            def norm_pair(re_ps, im_ps, re_out, im_out, width, bias_col=None):
                sq_re = nt.tile([P, width], F32, tag="nt", name="sqre")
                nc.scalar.activation(sq_re[:], re_ps[:], AF.Square,
                                     bias=bias_col if bias_col is not None
                                     else 0.0, scale=1.0)
                sq_im = nt.tile([P, width], F32, tag="nt", name="sqim")
                nc.scalar.activation(sq_im[:], im_ps[:], AF.Square,
                                     bias=0.0, scale=1.0)
                m = nt.tile([P, width], F32, tag="nt", name="m")
                nc.gpsimd.tensor_tensor(m[:], sq_re[:], sq_im[:], ALU.add)
                sq = nt.tile([P, width], F32, tag="nt", name="sq")
                nc.scalar.activation(sq[:], m[:], AF.Sqrt, bias=0.0, scale=1.0)
                n = nt.tile([P, width], F32, tag="nt", name="n")
                nc.vector.reciprocal_approx_fast(n[:], sq[:])
                if bias_col is not None:
                    nc.vector.scalar_tensor_tensor(
                        re_out, re_ps[:], bias_col, n[:], ALU.add, ALU.mult)
                else:
                    nc.vector.tensor_tensor(re_out, re_ps[:], n[:], ALU.mult)
                nc.vector.tensor_tensor(im_out, im_ps[:], n[:], ALU.mult)

            # ================= per-head pipeline =================
            for h in range(HPC):
                wq_b = w_cache.pop(("q", h))
                wk_b = w_cache.pop(("k", h))
                wv_b = w_cache.pop(("v", h))
                wo_r = w_cache.pop(("o", h))

                # ---- KV pass ----
                for ch in range(NCH):
                    chsl = slice(ch * CH, (ch + 1) * CH)
                    # V projection: out [t-tile, D]
                    for ts in range(CH // P):
                        tp0 = ch * CH + ts * P
                        pre = pools["ps"].tile([P, D], F32, tag="ps")
                        pim = pools["ps"].tile([P, D], F32, tag="ps")
                        for do in range(DS):
                            nc.tensor.matmul(
                                pre[:], lhsT=kv_cos[:, do, tp0:tp0 + P],
                                rhs=wv_b[:, do, :], start=(do == 0), stop=False)
                        nc.tensor.matmul(
                            pre[:], lhsT=ones_bf[0:1, :P], rhs=bv_rows[h][:],
                            start=False, stop=True)
                        for do in range(DS):
                            nc.tensor.matmul(
                                pim[:], lhsT=kv_sin[:, do, tp0:tp0 + P],
                                rhs=wv_b[:, do, :], start=(do == 0),
                                stop=(do == DS - 1))
                        trow = ch * (CH // P) + ts
                        norm_pair(pre, pim, v_re[:, trow, :], v_im[:, trow, :],
                                  D)
                    # K projection: out [D'-slice, t]
                    for dso in range(DS):
                        pre = pools["ps"].tile([P, CH], F32, tag="ps")
                        pim = pools["ps"].tile([P, CH], F32, tag="ps")
                        for do in range(DS):
                            nc.tensor.matmul(
                                pre[:], lhsT=wk_b[:, do, dso * P:(dso + 1) * P],
                                rhs=kv_cos[:, do, chsl], start=(do == 0),
                                stop=(do == DS - 1))
                        for do in range(DS):
                            nc.tensor.matmul(
                                pim[:], lhsT=wk_b[:, do, dso * P:(dso + 1) * P],
                                rhs=kv_sin[:, do, chsl], start=(do == 0),
                                stop=(do == DS - 1))
                        norm_pair(pre, pim, kt_re[:, dso, chsl],
                                  kt_im[:, dso, chsl], CH,
                                  bias_col=bkc[h][:, dso:dso + 1])

                # prefetch next head's weights (DMA + Pool cast)
                if h + 1 < HPC:
                    w_cache[("q", h + 1)] = load_w(WQ[h + 1], F16, "wq")
                    w_cache[("k", h + 1)] = load_w(WK[h + 1], F16, "wk")
                    w_cache[("v", h + 1)] = load_w(WV[h + 1], F16, "wv")
                    w_cache[("o", h + 1)] = load_w(
                        WO[(h + 1) * D:(h + 2) * D, :], F16, "wo")

                # ---- Q pass ----
                for ch in range(NCH):
                    qsl = slice(ch * CH, (ch + 1) * CH)
                    qt_re = pools["qt"].tile([P, DS, CH], F16, tag="qtre")
                    qt_im = pools["qt"].tile([P, DS, CH], F16, tag="qtim")
                    for dso in range(DS):
                        pre = pools["ps"].tile([P, CH], F32, tag="ps")
                        pim = pools["ps"].tile([P, CH], F32, tag="ps")
                        for do in range(DS):
                            nc.tensor.matmul(
                                pre[:], lhsT=wq_b[:, do, dso * P:(dso + 1) * P],
                                rhs=q_cos[:, do, qsl], start=(do == 0),
                                stop=(do == DS - 1))
                        for do in range(DS):
                            nc.tensor.matmul(
                                pim[:], lhsT=wq_b[:, do, dso * P:(dso + 1) * P],
                                rhs=q_sin[:, do, qsl], start=(do == 0),
                                stop=(do == DS - 1))
                        norm_pair(pre, pim, qt_re[:, dso, :], qt_im[:, dso, :],
                                  CH, bias_col=bqc[h][:, dso:dso + 1])

                    # scores + exp -> P^T [Tkv, tq-chunk]
                    pt_all = pools["pt"].tile([P, T // P, CH], F16, tag="pt")
                    for to in range(T // P):
                        ps_s = pools["ps"].tile([P, CH], F32, tag="ps")
                        for do in range(DS):
                            nc.tensor.matmul(
                                ps_s[:], lhsT=kt_re[:, do, to * P:(to + 1) * P],
                                rhs=qt_re[:, do, :], start=(do == 0), stop=False)
                        for do in range(DS):
                            nc.tensor.matmul(
                                ps_s[:], lhsT=kt_im[:, do, to * P:(to + 1) * P],
                                rhs=qt_im[:, do, :], start=False,
                                stop=(do == DS - 1))
                        nc.scalar.activation(pt_all[:, to, :], ps_s[:], AF.Exp,
                                             bias=0.0, scale=1.0 / D)

                    # PV: O^T [D'-slice, tq-chunk], 2 groups of 2 D'-slices
                    oh_re = pools["oh"].tile([P, DS, CH], F16, tag="ohre")
                    oh_im = pools["oh"].tile([P, DS, CH], F16, tag="ohim")
                    for grp in range(2):
                        ps_tiles = {}
                        for dso in (2 * grp, 2 * grp + 1):
                            ps_tiles[(dso, 0)] = pools["ps"].tile(
                                [P, CH], F32, tag="ps", name=f"pv{dso}re")
                            ps_tiles[(dso, 1)] = pools["ps"].tile(
                                [P, CH], F32, tag="ps", name=f"pv{dso}im")
                        for to in range(T // P):
                            for dso in (2 * grp, 2 * grp + 1):
                                nc.tensor.matmul(
                                    ps_tiles[(dso, 0)][:],
                                    lhsT=v_re[:, to, dso * P:(dso + 1) * P],
                                    rhs=pt_all[:, to, :], start=(to == 0),
                                    stop=(to == T // P - 1))
                                nc.tensor.matmul(
                                    ps_tiles[(dso, 1)][:],
                                    lhsT=v_im[:, to, dso * P:(dso + 1) * P],
                                    rhs=pt_all[:, to, :], start=(to == 0),
                                    stop=(to == T // P - 1))
                        for dso in (2 * grp, 2 * grp + 1):
                            norm_pair(ps_tiles[(dso, 0)], ps_tiles[(dso, 1)],
                                      oh_re[:, dso, :], oh_im[:, dso, :], CH)

                    # final dense partial: Z [tq, D] += Ohat^T.T @ wo_h
                    for ts in range(CH // P):
                        u = ch * (CH // P) + ts   # 128-row block index in T
                        pzre = pools["ps"].tile([P, D], F32, tag="ps")
                        pzim = pools["ps"].tile([P, D], F32, tag="ps")
                        for do in range(DS):
                            nc.tensor.matmul(
                                pzre[:], lhsT=oh_re[:, do, ts * P:(ts + 1) * P],
                                rhs=wo_r[:, do, :], start=(do == 0),
                                stop=(do == DS - 1))
                        for do in range(DS):
                            nc.tensor.matmul(
                                pzim[:], lhsT=oh_im[:, do, ts * P:(ts + 1) * P],
                                rhs=wo_r[:, do, :], start=(do == 0),
                                stop=(do == DS - 1))
                        if h == 0:
                            # fold bo/4 into the drain
                            nc.vector.tensor_tensor(z_re[:, u, :], pzre[:],
                                                    bo8[:], ALU.add)
                            nc.scalar.activation(z_im[:, u, :], pzim[:],
                                                 AF.Copy, bias=0.0, scale=1.0)
                        else:
                            zs_re = pools["zs"].tile([P, D], F32, tag="zs",
                                                     name=f"zsre{u}")
                            zs_im = pools["zs"].tile([P, D], F32, tag="zs",
                                                     name=f"zsim{u}")
                            nc.vector.tensor_tensor(zs_re[:], pzre[:],
                                                    z_re[:, u, :], ALU.add)
                            nc.vector.tensor_tensor(zs_im[:], pzim[:],
                                                    z_im[:, u, :], ALU.add)
                            # zb row layout: rank g*128 + [re 64 | im 64]
                            qq = u // 2
                            su = u % 2
                            zb = zbs[qq]
                            g0 = 2 * su
                            nc.sync.dma_start(
                                zb[g0 * P: g0 * P + 64, :], zs_re[0:64, :])
                            nc.sync.dma_start(
                                zb[g0 * P + 64: g0 * P + 128, :], zs_im[0:64, :])
                            nc.sync.dma_start(
                                zb[(g0 + 1) * P: (g0 + 1) * P + 64, :],
                                zs_re[64:128, :])
                            nc.sync.dma_start(
                                zb[(g0 + 1) * P + 64: (g0 + 1) * P + 128, :],
                                zs_im[64:128, :])

                    if h == HPC - 1:
                        for qq in (2 * ch, 2 * ch + 1):
                            nc.gpsimd.collective_compute(
                                "ReduceScatter", ALU.add,
                                replica_groups=[[0, 1, 2, 3], [4, 5, 6, 7]],
                                ins=[zbs[qq].opt()],
                                outs=[rs_outs[qq].opt()],
                            )

            # ======== atan2(zim, zre)/pi, quarters batched in pairs ========
            for pp in range(2):
                qa, qb = 2 * pp, 2 * pp + 1
                zre_t = pools["z"].tile([P, D], F32, tag="z", name=f"zre{pp}")
                nc.sync.dma_start(zre_t[0:64, :], rs_outs[qa][0:64, :])
                nc.sync.dma_start(zre_t[64:128, :], rs_outs[qb][0:64, :])
                zim_t = pools["z"].tile([P, D], F32, tag="z", name=f"zim{pp}")
                nc.sync.dma_start(zim_t[0:64, :], rs_outs[qa][64:128, :])
                nc.sync.dma_start(zim_t[64:128, :], rs_outs[qb][64:128, :])
                zre, zim = zre_t[:, :], zim_t[:, :]

                def ft(nm, pp=pp):
                    return nt.tile([P, D], F32, tag="nt", name=f"{nm}{pp}")
                t1 = ft("f1")
                nc.scalar.activation(t1[:], zre, AF.Square, bias=0.0, scale=1.0)
                t2 = ft("f2")
                nc.vector.tensor_tensor(t2[:], zim, zim, ALU.mult)
                m = ft("f3")
                nc.vector.tensor_tensor(m[:], t1[:], t2[:], ALU.add)
                az = ft("f5")
                nc.scalar.activation(az[:], m[:], AF.Sqrt, bias=0.0, scale=1.0)
                den1 = ft("f6")
                nc.vector.tensor_tensor(den1[:], az[:], zre, ALU.add)
                r1 = ft("f7")
                nc.vector.reciprocal_approx_fast(r1[:], den1[:])
                ta0 = ft("f8")
                nc.vector.tensor_tensor(ta0[:], zim, r1[:], ALU.mult)
                ta = ft("f9")
                nc.vector.tensor_scalar(ta[:], ta0[:], 1e8, -1e8, ALU.min,
                                        ALU.max)
                num2 = ft("fa")
                nc.vector.tensor_tensor(num2[:], az[:], zre, ALU.subtract)
                r2 = ft("fb")
                nc.vector.reciprocal_approx_fast(r2[:], zim)
                tb0 = ft("fc")
                nc.vector.tensor_tensor(tb0[:], num2[:], r2[:], ALU.mult)
                tb = ft("fd")
                nc.vector.tensor_scalar(tb[:], tb0[:], 1e8, -1e8, ALU.min,
                                        ALU.max)
                ata = ft("fe")
                nc.scalar.activation(ata[:], ta[:], AF.Arctan, bias=0.0,
                                     scale=1.0)
                atb = ft("ff")
                nc.scalar.activation(atb[:], tb[:], AF.Arctan, bias=0.0,
                                     scale=1.0)
                mask = ft("fg")
                nc.vector.tensor_scalar(mask[:], zre, 0.0, None, ALU.is_ge)
                dsel = ft("fh")
                nc.vector.tensor_tensor(dsel[:], ata[:], atb[:], ALU.subtract)
                md = ft("fi")
                nc.vector.tensor_tensor(md[:], mask[:], dsel[:], ALU.mult)
                sel = ft("fj")
                nc.vector.tensor_tensor(sel[:], atb[:], md[:], ALU.add)
                outt = ft("fk")
                nc.vector.tensor_scalar(outt[:], sel[:], 2.0 / PI, None,
                                        ALU.mult)
                nc.sync.dma_start(OUT[qa * 64:(qa + 1) * 64, :], outt[0:64, :])
                nc.sync.dma_start(OUT[qb * 64:(qb + 1) * 64, :], outt[64:128, :])

    nc.finalize()
    return nc


_NC_CACHE = {}


def _get_nc():
    if "nc" not in _NC_CACHE:
        _NC_CACHE["nc"] = build()
    return _NC_CACHE["nc"]


def kernel(**inputs):
    query = np.ascontiguousarray(np.asarray(inputs["query"], dtype=np.float32))
    keyvalue = np.ascontiguousarray(np.asarray(inputs["keyvalue"], dtype=np.float32))
    wq = np.asarray(inputs["wq"], dtype=np.float32)
    wk = np.asarray(inputs["wk"], dtype=np.float32)
    wv = np.asarray(inputs["wv"], dtype=np.float32)
    bq = np.asarray(inputs["bq"], dtype=np.float32)
    bk = np.asarray(inputs["bk"], dtype=np.float32)
    bv = np.asarray(inputs["bv"], dtype=np.float32)
    wo = np.asarray(inputs["wo"], dtype=np.float32)
    bo = np.asarray(inputs["bo"], dtype=np.float32)

    in_maps = []
    for c in range(N_CORES):
        b, g = c // 4, c % 4
        h0 = g * HPC
        in_maps.append({
            "query": query[b],
            "keyvalue": keyvalue[b],
            "wq": np.ascontiguousarray(wq[h0:h0 + HPC]),
            "wk": np.ascontiguousarray(wk[h0:h0 + HPC]),
            "wv": np.ascontiguousarray(wv[h0:h0 + HPC]),
            "bq": np.ascontiguousarray(bq[h0:h0 + HPC]),
            "bk": np.ascontiguousarray(bk[h0:h0 + HPC]),
            "bv": np.ascontiguousarray(bv[h0:h0 + HPC]),
            "wo": np.ascontiguousarray(wo[h0 * D:(h0 + HPC) * D]),
            "bo": bo,
        })

    nc = _get_nc()
    res = run_bass_kernel_spmd(nc, in_maps, core_ids=list(range(N_CORES)))
    _NC_CACHE["last_results"] = res
    out = np.empty((B, T, D), np.float32)
    for c in range(N_CORES):
        b, g = c // 4, c % 4
        o = res.results[c]["out"]          # [256, 512]: 4 quarters x 64 rows
        for qq in range(4):
            out[b, qq * 256 + g * 64: qq * 256 + (g + 1) * 64, :] = \
                o[qq * 64:(qq + 1) * 64, :]
    return out


# revision 21
# speedup vs baseline: 1.1071x; 1.0740x over previous
"""Distributed Trainium2 Bass kernel for the phasor attention problem
(nn_Attention_17798344475248).

Sharding: 8 cores = 2 batches x 4 head-groups (2 heads each). Each core
computes its batch's Q/K/V projections for its 2 heads, phasor attention,
and a partial final-dense output; partials are summed with 4 pipelined
4-rank ReduceScatters per batch group; each core finishes atan2 on its
4x64-row slices of the output.

v3 design notes:
- all encodes + K/V/z SBUF-resident (no DRAM staging round trips)
- fp16 storage for every PE operand: quantization noise stays below the
  branch-cut flip threshold (validated against a numpy quantization sim);
  fp16 multiplies are exact into f32 PSUM, unlike f32r
- each projection's re/im PSUM pair is ONE [128,1024] tile: the norm's
  Square runs once over both halves (ACT was the drain bottleneck)
- norm n = 1/sqrt(re^2+im^2) via ACT Square/Sqrt + DVE fast reciprocal;
  Sqrt<->Exp act-table switches only at Q-pass phase boundaries
- z accumulated in SBUF across heads; zb written once, ReduceScatter
  fired per tq-quarter as head 1 completes it
- emission order kv-enc -> h0 KV -> q-enc -> h0 Q keeps PE fed during
  the encode phase
- QCH=512 (max moving dim): fewer, bigger instructions everywhere
"""
import sys

sys.path.insert(0, "/opt/trn_rl_repo")

import numpy as np

import concourse.bass as bass
import concourse.tile as tile
from concourse import bacc, mybir
from concourse.bass_utils import run_bass_kernel_spmd
from concourse.masks import make_identity

F32 = mybir.dt.float32
F32R = mybir.dt.float32r
F16 = mybir.dt.float16
AF = mybir.ActivationFunctionType
ALU = mybir.AluOpType
PI = float(np.pi)

B, T, D, H = 2, 1024, 512, 8
P = 128
DS = D // P          # 4 partition-slices of the model dim
CH = 512             # chunk width along t (both kv and q passes)
NCH = T // CH        # 2
N_CORES = 8
HPC = 2              # heads per core


def build(debug=False):
    nc = bacc.Bacc("TRN2", target_bir_lowering=False, debug=False,
                   num_devices=N_CORES)
    cpi2 = nc.alloc_sbuf_tensor("const-f32-pi2", [P, 1], F32)
    nc.gpsimd.memset(cpi2.ap(), PI / 2)
    nc.const_aps.aps[(F32, PI / 2)] = cpi2.ap()
    nc.all_engine_barrier()

    # ---- I/O ----
    QUERY = nc.dram_tensor("query", [T, D], F32, kind="ExternalInput")
    KEYVALUE = nc.dram_tensor("keyvalue", [T, D], F32, kind="ExternalInput")
    WQ = nc.dram_tensor("wq", [HPC, D, D], F32, kind="ExternalInput")
    WK = nc.dram_tensor("wk", [HPC, D, D], F32, kind="ExternalInput")
    WV = nc.dram_tensor("wv", [HPC, D, D], F32, kind="ExternalInput")
    BQ = nc.dram_tensor("bq", [HPC, D], F32, kind="ExternalInput")
    BK = nc.dram_tensor("bk", [HPC, D], F32, kind="ExternalInput")
    BV = nc.dram_tensor("bv", [HPC, D], F32, kind="ExternalInput")
    WO = nc.dram_tensor("wo", [HPC * D, D], F32, kind="ExternalInput")
    BO = nc.dram_tensor("bo", [D], F32, kind="ExternalInput")
    OUT = nc.dram_tensor("out", [T // 4, D], F32, kind="ExternalOutput")

    with tile.TileContext(nc) as tc:
        import contextlib
        with contextlib.ExitStack() as ctx:
            pools = {}
            for name, bufs, space in [
                ("persist", 1, "SBUF"),   # ident, z, bo8, ones, enc, ktv
                ("raw", 4, "SBUF"),
                ("nt", 6, "SBUF"),        # norm temps
                ("wf", 1, "SBUF"),        # f32 weight staging
                ("w", 1, "SBUF"),         # casted weights (per-head, tags)
                ("brow", 2, "SBUF"),      # bias staging rows
                ("qt", 1, "SBUF"), ("pt", 1, "SBUF"), ("oh", 1, "SBUF"),
                ("zs", 4, "SBUF"),        # h1 drain staging
                ("z", 2, "SBUF"),         # atan2 tiles
                ("pp", 3, "PSUM"),        # [128,1024] re|im pair tiles
                ("ps", 2, "PSUM"),        # [128,512] singles
                ("dram", 1, "DRAM"),
            ]:
                pools[name] = ctx.enter_context(
                    tc.tile_pool(name=name, bufs=bufs, space=space))

            persist = pools["persist"]
            nt = pools["nt"]
            ident = persist.tile([P, P], F32, tag="ident")
            make_identity(nc, ident[:])

            # ---- constant rows ----
            ones_f = persist.tile([1, CH], F32, tag="onesf")
            nc.vector.memset(ones_f[:], 1.0)
            ones_h = persist.tile([1, CH], F16, tag="onesh")
            nc.vector.tensor_copy(ones_h[:], ones_f[:])
            q128_f = persist.tile([1, P], F32, tag="q128f")
            nc.vector.memset(q128_f[:], 0.25)   # bo added once per core, 4 cores
            q128_r = persist.tile([1, P], F32R, tag="q128r")
            nc.vector.tensor_copy(q128_r[:], q128_f[:])
            bo_f = pools["brow"].tile([1, D], F32, tag="brow", name="bo_f")
            nc.sync.dma_start(bo_f[:], BO[:][None, :])
            bo_r = persist.tile([1, D], F32R, tag="bor")
            nc.vector.tensor_copy(bo_r[:], bo_f[:])
            # bo/4 broadcast to all partitions (folded into the h0 z drain)
            ps_bo = pools["ps"].tile([P, D], F32, tag="ps", name="ps_bo")
            nc.tensor.matmul(ps_bo[:], lhsT=q128_r[:], rhs=bo_r[:],
                             start=True, stop=True)
            bo8 = persist.tile([P, D], F32, tag="bo8")
            nc.vector.tensor_copy(bo8[:], ps_bo[:])

            # ---- per-head bias rows [1, D] in f16 ----
            def bias_row(SRC, h, nm):
                bf = pools["brow"].tile([1, D], F32, tag="brow",
                                        name=f"{nm}f{h}")
                nc.sync.dma_start(bf[:], SRC[h][None, :])
                br = persist.tile([1, D], F16, tag=f"{nm}{h}")
                nc.vector.tensor_copy(br[:], bf[:])
                return br

            bq_rows = [bias_row(BQ, h, "bq") for h in range(HPC)]
            bk_rows = [bias_row(BK, h, "bk") for h in range(HPC)]
            bv_rows = [bias_row(BV, h, "bv") for h in range(HPC)]

            # ---- weights (per-head tags, bufs=1: h1 load waits h0 users) ----
            def load_w(W_ap, tag):
                wf = pools["wf"].tile([P, DS, D], F32, tag="wf")
                nc.sync.dma_start(wf[:], W_ap.rearrange("(o p) D -> p o D", p=P))
                w = pools["w"].tile([P, DS, D], F16, tag=tag)
                nc.vector.tensor_copy(w[:], wf[:])
                return w

            def load_head_weights(h):
                return {
                    "q": load_w(WQ[h], "wq"),
                    "k": load_w(WK[h], "wk"),
                    "v": load_w(WV[h], "wv"),
                    "o": load_w(WO[h * D:(h + 1) * D, :], "wo"),
                }

            # ---- persistent state ----
            kv_cos = persist.tile([P, DS, T], F16, tag="kvcos")
            kv_sin = persist.tile([P, DS, T], F16, tag="kvsin")
            q_cos = persist.tile([P, DS, T], F16, tag="qcos")
            q_sin = persist.tile([P, DS, T], F16, tag="qsin")
            kt_re = persist.tile([P, DS, T], F16, tag="ktre")
            kt_im = persist.tile([P, DS, T], F16, tag="ktim")
            v_re = persist.tile([P, T // P, D], F16, tag="vre")
            v_im = persist.tile([P, T // P, D], F16, tag="vim")
            z_re = persist.tile([P, T // P, D], F32, tag="zre")
            z_im = persist.tile([P, T // P, D], F32, tag="zim")

            dram = pools["dram"]
            zbs = [dram.tile([4 * P, D], F32, name=f"zb{q}") for q in range(4)]
            rs_outs = [dram.tile([P, D], F32, name=f"rsout{q}")
                       for q in range(4)]

            w_cache = {0: load_head_weights(0)}

            # ---- phasor encode: x -> (cos pi x, sin pi x), [d, t] layout ----
            def encode(src_dram, cos_t, sin_t):
                for ch in range(NCH):
                    chsl = slice(ch * CH, (ch + 1) * CH)
                    raw_tiles = []
                    for ts in range(CH // P):
                        rt = pools["raw"].tile([P, D], F32, tag="raw")
                        nc.sync.dma_start(
                            rt[:],
                            src_dram[ch * CH + ts * P: ch * CH + (ts + 1) * P, :])
                        raw_tiles.append(rt)
                    for ds in range(DS):
                        pt_ = pools["ps"].tile([P, CH], F32, tag="ps")
                        for ts in range(CH // P):
                            nc.tensor.transpose(
                                pt_[:, ts * P:(ts + 1) * P],
                                raw_tiles[ts][:, ds * P:(ds + 1) * P], ident[:])
                        nc.scalar.activation(sin_t[:, ds, chsl], pt_[:], AF.Sin,
                                             bias=0.0, scale=PI)
                        ab = nt.tile([P, CH], F32, tag="nt")
                        nc.scalar.activation(ab[:], pt_[:], AF.Abs,
                                             bias=0.0, scale=1.0)
                        nc.scalar.activation(cos_t[:, ds, chsl], ab[:], AF.Sin,
                                             bias=PI / 2, scale=-PI)

            # ---- norm: (re,im) pair psum [128, 2W] -> unit modulus ----
            def norm_pair(pp, re_out, im_out, width):
                sq = nt.tile([P, 2 * width], F32, tag="nt2", name="sq", bufs=3)
                nc.scalar.activation(sq[:], pp[:, :], AF.Square,
                                     bias=0.0, scale=1.0)
                m = nt.tile([P, width], F32, tag="nt", name="m")
                nc.gpsimd.tensor_tensor(m[:], sq[:, 0:width],
                                        sq[:, width:2 * width], ALU.add)
                sqm = nt.tile([P, width], F32, tag="nt", name="sqm")
                nc.scalar.activation(sqm[:], m[:], AF.Sqrt, bias=0.0, scale=1.0)
                n = nt.tile([P, width], F32, tag="nt", name="n")
                nc.vector.reciprocal_approx_fast(n[:], sqm[:])
                nc.vector.tensor_tensor(re_out, pp[:, 0:width], n[:], ALU.mult)
                nc.vector.tensor_tensor(im_out, pp[:, width:2 * width], n[:],
                                        ALU.mult)

            # ================= per-head passes =================
            def kv_pass(h, w):
                for ch in range(NCH):
                    chsl = slice(ch * CH, (ch + 1) * CH)
                    # V projection: out [t-tile, D]
                    for ts in range(CH // P):
                        tp0 = ch * CH + ts * P
                        pp = pools["pp"].tile([P, 2 * D], F32, tag="pp")
                        for do in range(DS):
                            nc.tensor.matmul(
                                pp[:, 0:D], lhsT=kv_cos[:, do, tp0:tp0 + P],
                                rhs=w["v"][:, do, :], start=(do == 0),
                                stop=False)
                        nc.tensor.matmul(
                            pp[:, 0:D], lhsT=ones_h[0:1, :P], rhs=bv_rows[h][:],
                            start=False, stop=True)
                        for do in range(DS):
                            nc.tensor.matmul(
                                pp[:, D:2 * D], lhsT=kv_sin[:, do, tp0:tp0 + P],
                                rhs=w["v"][:, do, :], start=(do == 0),
                                stop=(do == DS - 1))
                        trow = ch * (CH // P) + ts
                        norm_pair(pp, v_re[:, trow, :], v_im[:, trow, :], D)
                    # K projection: out [D'-slice, t] (+ bias row matmul)
                    for dso in range(DS):
                        pp = pools["pp"].tile([P, 2 * CH], F32, tag="pp")
                        for do in range(DS):
                            nc.tensor.matmul(
                                pp[:, 0:CH],
                                lhsT=w["k"][:, do, dso * P:(dso + 1) * P],
                                rhs=kv_cos[:, do, chsl], start=(do == 0),
                                stop=False)
                        nc.tensor.matmul(
                            pp[:, 0:CH],
                            lhsT=bk_rows[h][0:1, dso * P:(dso + 1) * P],
                            rhs=ones_h[:], start=False, stop=True)
                        for do in range(DS):
                            nc.tensor.matmul(
                                pp[:, CH:2 * CH],
                                lhsT=w["k"][:, do, dso * P:(dso + 1) * P],
                                rhs=kv_sin[:, do, chsl], start=(do == 0),
                                stop=(do == DS - 1))
                        norm_pair(pp, kt_re[:, dso, chsl], kt_im[:, dso, chsl],
                                  CH)

            def q_pass(h, w):
                for ch in range(NCH):
                    qsl = slice(ch * CH, (ch + 1) * CH)
                    qt_re = pools["qt"].tile([P, DS, CH], F16, tag="qtre")
                    qt_im = pools["qt"].tile([P, DS, CH], F16, tag="qtim")
                    for dso in range(DS):
                        pp = pools["pp"].tile([P, 2 * CH], F32, tag="pp")
                        for do in range(DS):
                            nc.tensor.matmul(
                                pp[:, 0:CH],
                                lhsT=w["q"][:, do, dso * P:(dso + 1) * P],
                                rhs=q_cos[:, do, qsl], start=(do == 0),
                                stop=False)
                        nc.tensor.matmul(
                            pp[:, 0:CH],
                            lhsT=bq_rows[h][0:1, dso * P:(dso + 1) * P],
                            rhs=ones_h[:], start=False, stop=True)
                        for do in range(DS):
                            nc.tensor.matmul(
                                pp[:, CH:2 * CH],
                                lhsT=w["q"][:, do, dso * P:(dso + 1) * P],
                                rhs=q_sin[:, do, qsl], start=(do == 0),
                                stop=(do == DS - 1))
                        norm_pair(pp, qt_re[:, dso, :], qt_im[:, dso, :], CH)

                    # scores + exp -> P^T [Tkv, tq-chunk]
                    pt_all = pools["pt"].tile([P, T // P, CH], F16, tag="pt")
                    for to in range(T // P):
                        ps_s = pools["ps"].tile([P, CH], F32, tag="ps")
                        for do in range(DS):
                            nc.tensor.matmul(
                                ps_s[:], lhsT=kt_re[:, do, to * P:(to + 1) * P],
                                rhs=qt_re[:, do, :], start=(do == 0), stop=False)
                        for do in range(DS):
                            nc.tensor.matmul(
                                ps_s[:], lhsT=kt_im[:, do, to * P:(to + 1) * P],
                                rhs=qt_im[:, do, :], start=False,
                                stop=(do == DS - 1))
                        nc.scalar.activation(pt_all[:, to, :], ps_s[:], AF.Exp,
                                             bias=0.0, scale=1.0 / D)

                    # PV: O^T [D'-slice, tq-chunk], 2 groups of 2 D'-slices
                    oh_re = pools["oh"].tile([P, DS, CH], F16, tag="ohre")
                    oh_im = pools["oh"].tile([P, DS, CH], F16, tag="ohim")
                    for grp in range(2):
                        pps = {}
                        for dso in (2 * grp, 2 * grp + 1):
                            pps[dso] = pools["pp"].tile(
                                [P, 2 * CH], F32, tag="pp", name=f"pv{dso}")
                        for to in range(T // P):
                            for dso in (2 * grp, 2 * grp + 1):
                                nc.tensor.matmul(
                                    pps[dso][:, 0:CH],
                                    lhsT=v_re[:, to, dso * P:(dso + 1) * P],
                                    rhs=pt_all[:, to, :], start=(to == 0),
                                    stop=(to == T // P - 1))
                                nc.tensor.matmul(
                                    pps[dso][:, CH:2 * CH],
                                    lhsT=v_im[:, to, dso * P:(dso + 1) * P],
                                    rhs=pt_all[:, to, :], start=(to == 0),
                                    stop=(to == T // P - 1))
                        for dso in (2 * grp, 2 * grp + 1):
                            norm_pair(pps[dso], oh_re[:, dso, :],
                                      oh_im[:, dso, :], CH)

                    # final dense partial: Z [tq, D] += Ohat^T.T @ wo_h
                    for ts in range(CH // P):
                        u = ch * (CH // P) + ts   # 128-row block index in T
                        pp = pools["pp"].tile([P, 2 * D], F32, tag="pp",
                                              name=f"fd{u}")
                        for do in range(DS):
                            nc.tensor.matmul(
                                pp[:, 0:D],
                                lhsT=oh_re[:, do, ts * P:(ts + 1) * P],
                                rhs=w["o"][:, do, :], start=(do == 0),
                                stop=(do == DS - 1))
                        for do in range(DS):
                            nc.tensor.matmul(
                                pp[:, D:2 * D],
                                lhsT=oh_im[:, do, ts * P:(ts + 1) * P],
                                rhs=w["o"][:, do, :], start=(do == 0),
                                stop=(do == DS - 1))
                        if h == 0:
                            # fold bo/4 into the drain
                            nc.vector.tensor_tensor(z_re[:, u, :], pp[:, 0:D],
                                                    bo8[:], ALU.add)
                            nc.scalar.activation(z_im[:, u, :], pp[:, D:2 * D],
                                                 AF.Copy, bias=0.0, scale=1.0)
                        else:
                            zs_re = pools["zs"].tile([P, D], F32, tag="zs",
                                                     name=f"zsre{u}")
                            zs_im = pools["zs"].tile([P, D], F32, tag="zs",
                                                     name=f"zsim{u}")
                            nc.vector.tensor_tensor(zs_re[:], pp[:, 0:D],
                                                    z_re[:, u, :], ALU.add)
                            nc.vector.tensor_tensor(zs_im[:], pp[:, D:2 * D],
                                                    z_im[:, u, :], ALU.add)
                            # zb row layout: rank g*128 + [re 64 | im 64]
                            qq = u // 2
                            su = u % 2
                            zb = zbs[qq]
                            g0 = 2 * su
                            nc.sync.dma_start(
                                zb[g0 * P: g0 * P + 64, :], zs_re[0:64, :])
                            nc.sync.dma_start(
                                zb[g0 * P + 64: g0 * P + 128, :], zs_im[0:64, :])
                            nc.sync.dma_start(
                                zb[(g0 + 1) * P: (g0 + 1) * P + 64, :],
                                zs_re[64:128, :])
                            nc.sync.dma_start(
                                zb[(g0 + 1) * P + 64: (g0 + 1) * P + 128, :],
                                zs_im[64:128, :])

                    if h == HPC - 1:
                        for qq in (2 * ch, 2 * ch + 1):
                            nc.gpsimd.collective_compute(
                                "ReduceScatter", ALU.add,
                                replica_groups=[[0, 1, 2, 3], [4, 5, 6, 7]],
                                ins=[zbs[qq].opt()],
                                outs=[rs_outs[qq].opt()],
                            )

            # ================= schedule =================
            encode(KEYVALUE, kv_cos, kv_sin)
            for h in range(HPC):
                w = w_cache.pop(h)
                kv_pass(h, w)
                if h == 0:
                    encode(QUERY, q_cos, q_sin)
                    w_cache[1] = load_head_weights(1)
                q_pass(h, w)

            # ======== atan2(zim, zre)/pi, quarters batched in pairs ========
            for pp_ in range(2):
                qa, qb = 2 * pp_, 2 * pp_ + 1
                zre_t = pools["z"].tile([P, D], F32, tag="z", name=f"zre{pp_}")
                nc.sync.dma_start(zre_t[0:64, :], rs_outs[qa][0:64, :])
                nc.sync.dma_start(zre_t[64:128, :], rs_outs[qb][0:64, :])
                zim_t = pools["z"].tile([P, D], F32, tag="z", name=f"zim{pp_}")
                nc.sync.dma_start(zim_t[0:64, :], rs_outs[qa][64:128, :])
                nc.sync.dma_start(zim_t[64:128, :], rs_outs[qb][64:128, :])
                zre, zim = zre_t[:, :], zim_t[:, :]

                def ft(nm, pp_=pp_):
                    return nt.tile([P, D], F32, tag="nt", name=f"{nm}{pp_}")
                t1 = ft("f1")
                nc.scalar.activation(t1[:], zre, AF.Square, bias=0.0, scale=1.0)
                t2 = ft("f2")
                nc.vector.tensor_tensor(t2[:], zim, zim, ALU.mult)
                m = ft("f3")
                nc.vector.tensor_tensor(m[:], t1[:], t2[:], ALU.add)
                az = ft("f5")
                nc.scalar.activation(az[:], m[:], AF.Sqrt, bias=0.0, scale=1.0)
                den1 = ft("f6")
                nc.vector.tensor_tensor(den1[:], az[:], zre, ALU.add)
                r1 = ft("f7")
                nc.vector.reciprocal_approx_fast(r1[:], den1[:])
                ta0 = ft("f8")
                nc.vector.tensor_tensor(ta0[:], zim, r1[:], ALU.mult)
                ta = ft("f9")
                nc.vector.tensor_scalar(ta[:], ta0[:], 1e8, -1e8, ALU.min,
                                        ALU.max)
                num2 = ft("fa")
                nc.vector.tensor_tensor(num2[:], az[:], zre, ALU.subtract)
                r2 = ft("fb")
                nc.vector.reciprocal_approx_fast(r2[:], zim)
                tb0 = ft("fc")
                nc.vector.tensor_tensor(tb0[:], num2[:], r2[:], ALU.mult)
                tb = ft("fd")
                nc.vector.tensor_scalar(tb[:], tb0[:], 1e8, -1e8, ALU.min,
                                        ALU.max)
                ata = ft("fe")
                nc.scalar.activation(ata[:], ta[:], AF.Arctan, bias=0.0,
                                     scale=1.0)
                atb = ft("ff")
                nc.scalar.activation(atb[:], tb[:], AF.Arctan, bias=0.0,
                                     scale=1.0)
                mask = ft("fg")
                nc.vector.tensor_scalar(mask[:], zre, 0.0, None, ALU.is_ge)
                dsel = ft("fh")
                nc.vector.tensor_tensor(dsel[:], ata[:], atb[:], ALU.subtract)
                md = ft("fi")
                nc.vector.tensor_tensor(md[:], mask[:], dsel[:], ALU.mult)
                sel = ft("fj")
                nc.vector.tensor_tensor(sel[:], atb[:], md[:], ALU.add)
                outt = ft("fk")
                nc.vector.tensor_scalar(outt[:], sel[:], 2.0 / PI, None,
                                        ALU.mult)
                nc.sync.dma_start(OUT[qa * 64:(qa + 1) * 64, :], outt[0:64, :])
                nc.sync.dma_start(OUT[qb * 64:(qb + 1) * 64, :], outt[64:128, :])

    nc.finalize()
    return nc


_NC_CACHE = {}


def _get_nc():
    if "nc" not in _NC_CACHE:
        _NC_CACHE["nc"] = build()
    return _NC_CACHE["nc"]


def kernel(**inputs):
    query = np.ascontiguousarray(np.asarray(inputs["query"], dtype=np.float32))
    keyvalue = np.ascontiguousarray(np.asarray(inputs["keyvalue"], dtype=np.float32))
    wq = np.asarray(inputs["wq"], dtype=np.float32)
    wk = np.asarray(inputs["wk"], dtype=np.float32)
    wv = np.asarray(inputs["wv"], dtype=np.float32)
    bq = np.asarray(inputs["bq"], dtype=np.float32)
    bk = np.asarray(inputs["bk"], dtype=np.float32)
    bv = np.asarray(inputs["bv"], dtype=np.float32)
    wo = np.asarray(inputs["wo"], dtype=np.float32)
    bo = np.asarray(inputs["bo"], dtype=np.float32)

    in_maps = []
    for c in range(N_CORES):
        b, g = c // 4, c % 4
        h0 = g * HPC
        in_maps.append({
            "query": query[b],
            "keyvalue": keyvalue[b],
            "wq": np.ascontiguousarray(wq[h0:h0 + HPC]),
            "wk": np.ascontiguousarray(wk[h0:h0 + HPC]),
            "wv": np.ascontiguousarray(wv[h0:h0 + HPC]),
            "bq": np.ascontiguousarray(bq[h0:h0 + HPC]),
            "bk": np.ascontiguousarray(bk[h0:h0 + HPC]),
            "bv": np.ascontiguousarray(bv[h0:h0 + HPC]),
            "wo": np.ascontiguousarray(wo[h0 * D:(h0 + HPC) * D]),
            "bo": bo,
        })

    nc = _get_nc()
    res = run_bass_kernel_spmd(nc, in_maps, core_ids=list(range(N_CORES)))
    _NC_CACHE["last_results"] = res
    out = np.empty((B, T, D), np.float32)
    for c in range(N_CORES):
        b, g = c // 4, c % 4
        o = res.results[c]["out"]          # [256, 512]: 4 quarters x 64 rows
        for qq in range(4):
            out[b, qq * 256 + g * 64: qq * 256 + (g + 1) * 64, :] = \
                o[qq * 64:(qq + 1) * 64, :]
    return out
